# revision 38
# baseline (speedup 1.0000x reference)
"""Trainium2 Bass kernel for nn_EquivariantMultiheadAttention.

Sharding: query-point axis (dim 1) split across 8 cores (16 points each),
per the sharding hint — no collectives needed.

Algorithmic restructure: the ky-MLP depends on just two scalars
(f_key, f_query), so exp(silu(ky3(f_q, f_k))) is approximated by 2D
Chebyshev interpolation (32 nodes/dim, max rel err ~6e-6).  That yields a
rank-32-per-channel factorization E_y = U @ Psi^T evaluated as one K=128
matmul per group-pair on the tensor engine — eliminating half of all SILU
work from the activation engine, which is the hard bottleneck (ACTIVATE is
1 elem/cycle/lane, dtype-independent).  mask*f_k / mask are folded into the
Psi factors so the softmax numerator/denominator come out of two matmuls +
fused DVE multiply-reduce against exp(silu(kg3)).

Device phase 1, software-pipelined per pair of tiles (each tile = one query
(b, q, sq) x 512 keys): kg-L1 matmul (K padded to 128: full-array stream) ->
SiLU -> kg-L2 dense block-diagonal [128x128] matmul -> SiLU -> L3
(zero-padded M=32 matmuls accumulating 32 tiles into one PSUM bank).
silu1 runs one pair ahead of silu2 so all PE work hides under ACT ops.
Phase 2 (Exp table): per group-pair: exp, fused affine_mul_reduce num/den,
batched normalization, residual + query mask.

TRN2 quirks handled: the PE clock gate (HAM) only un-throttles to 2.4 GHz
under sustained high-occupancy matmul activity and re-throttles if the PE
idles — dependency-free full-array filler matmuls keep the activity window
saturated for the whole of phase 1.

Final w_out projection happens host-side on the tiny [B,N,S,4] result.
"""
import numpy as np
import ml_dtypes

BF16 = ml_dtypes.bfloat16

B, N, S, DG, C, HID, COUT = 2, 128, 4, 8, 4, 32, 8
NCORE = 8
QL = N // NCORE          # 16 query points per core
KEY = N * S              # 512 keys
T = B * QL * S           # 128 tiles per core
M = 32                   # Chebyshev nodes per dim (= rank per channel)
NPAIR = 4                # group-pairs per core (32 tiles each)

_PROG = None


def _cheb_fit(inp):
    """Per (b, c): Chebyshev nodes + grid values G of exp(silu(ky3))."""
    cf = np.asarray(inp["coset_functions"], np.float64)
    kyW1 = np.asarray(inp["ky_W1"], np.float64)
    kyb1 = np.asarray(inp["ky_b1"], np.float64)
    kyW2 = np.asarray(inp["ky_W2"], np.float64)
    kyb2 = np.asarray(inp["ky_b2"], np.float64)
    kyW3 = np.asarray(inp["ky_W3"], np.float64)
    kyb3 = np.asarray(inp["ky_b3"], np.float64)

    def silu(x):
        return x / (1.0 + np.exp(-x))

    j = np.arange(M)
    cw = (-1.0) ** j * np.sin((2 * j + 1) * np.pi / (2 * M))
    nodes = np.zeros((B, C, M))
    G = np.zeros((B, C, M, M))
    for b in range(B):
        for c in range(C):
            f = cf[b, :, :, c].ravel()
            lo, hi = f.min(), f.max()
            x = (lo + hi) / 2 + (hi - lo) / 2 * np.cos((2 * j + 1) * np.pi / (2 * M))
            nodes[b, c] = x
            FK, FQ = np.meshgrid(x, x, indexing="ij")
            h = silu(FK[..., None] * kyW1[c, :, 0] + FQ[..., None] * kyW1[c, :, 1] + kyb1[c])
            h = silu(h @ kyW2[c].T + kyb2[c])
            G[b, c] = np.exp(silu(h @ kyW3[c].T + kyb3[c])[..., 0])  # [fk_node, fq_node]
    return nodes, G, cw


def _lag(x, nd, cw):
    """Barycentric Lagrange basis values: [len(x), M]."""
    d = x[:, None] - nd[None, :]
    ex = np.isclose(d, 0.0, atol=1e-12)
    d = np.where(ex, 1.0, d)
    L = cw[None, :] / d
    L = L / L.sum(1, keepdims=True)
    L = np.where(ex.any(1)[:, None], ex.astype(np.float64), L)
    return L


def _kept_indices(mask):
    """Per batch: indices of unmasked keys, padded (with idx 0) to a common
    multiple-of-8 length.  Padding columns get zero Psi weight, so dropping
    masked keys is exact."""
    kept = [np.flatnonzero(mask[b].reshape(-1)) for b in range(B)]
    keyc = int(np.ceil(max(len(k) for k in kept) / 8.0) * 8)
    keyc = min(keyc, KEY)
    pads = []
    valid = []
    for b in range(B):
        k = kept[b][:keyc]
        nv = len(k)
        pads.append(np.concatenate([k, np.zeros(keyc - nv, np.int64)]))
        valid.append(nv)
    return pads, valid, keyc


def _pack_globals(inp, kept, valid, keyc):
    cf = np.asarray(inp["coset_functions"], np.float64)
    mask = np.asarray(inp["mask"]).astype(np.float64)
    kgW1 = np.asarray(inp["kg_W1"], np.float32)
    kgW2 = np.asarray(inp["kg_W2"], np.float32)
    kgW3 = np.asarray(inp["kg_W3"], np.float32)
    out = {}
    # kg L1: lhsT [128, 128], rows 0-7 = g weights, row 8 = bias, rest 0
    # (K padded to 128 so the matmul streams through the full PE array).
    w1g = np.zeros((128, 128), np.float32)
    for c in range(C):
        w1g[0:DG, c * 32:(c + 1) * 32] = kgW1[c].T
    w1g[DG, :] = np.asarray(inp["kg_b1"], np.float32).reshape(128)
    out["w1g"] = w1g.astype(BF16)
    # kg L2: dense block-diagonal [128, 128] (full-array stream)
    w2full = np.zeros((128, 128), np.float32)
    for c in range(C):
        w2full[c * 32:(c + 1) * 32, c * 32:(c + 1) * 32] = kgW2[c].T
    out["w2full"] = w2full.astype(BF16)
    # kg L3: [128, 256], col 36s+c holds W3g[c] (s-slot packing)
    w3g = np.zeros((128, 256), np.float32)
    for s in range(8):
        for c in range(C):
            w3g[c * 32:(c + 1) * 32, 36 * s + c] = kgW3[c, 0, :]
    out["w3g"] = w3g.astype(BF16)
    # biases: col 0 = b2; col 4 = b3 pattern
    bias128 = np.zeros((128, 8), np.float32)
    bias128[:, 0] = np.asarray(inp["kg_b2"], np.float32).reshape(128)
    bias128[:, 4] = np.tile(np.asarray(inp["kg_b3"], np.float32).reshape(C), 32)
    out["bias128"] = bias128
    # Psi factors (num/den) per batch, key-compacted: [128, B*keyc]
    nodes, G, cw = _cheb_fit(inp)
    psin = np.zeros((128, B * keyc), np.float64)
    psid = np.zeros((128, B * keyc), np.float64)
    for b in range(B):
        ki = kept[b]
        nv = valid[b]
        mk = mask[b].ravel()[ki]
        mk[nv:] = 0.0                                  # padding -> zero weight
        for c in range(C):
            fk = cf[b, :, :, c].ravel()[ki]
            Lk = _lag(fk, nodes[b, c], cw)            # [keyc, M]
            psin[32 * c:32 * c + 32, b * keyc:(b + 1) * keyc] = (Lk * (mk * fk)[:, None]).T
            psid[32 * c:32 * c + 32, b * keyc:(b + 1) * keyc] = (Lk * mk[:, None]).T
    out["psin2"] = psin.astype(BF16)
    out["psid2"] = psid.astype(BF16)
    return out, (nodes, G, cw)


def _pack_core(core, inp, aux, kept, keyc):
    nodes, G, cw = aux
    g = np.asarray(inp["pairwise_g"], np.float32)
    cf = np.asarray(inp["coset_functions"], np.float64)
    mask = np.asarray(inp["mask"]).astype(np.float32)
    qs = slice(core * QL, (core + 1) * QL)
    out = {}
    # g tiles, pair-ordered: pair p = tiles (t, t+4), t = 8*(p//4) + p%4
    gt = g[:, qs]                                        # [B,QL,N,S,S,DG]
    g_t = np.zeros((T, DG + 1, keyc), np.float32)
    gfull = gt.transpose(0, 1, 3, 5, 2, 4).reshape(B, T // B, DG, KEY)
    for b in range(B):
        g_t[b * (T // B):(b + 1) * (T // B), 0:DG, :] = gfull[b][:, :, kept[b]]
    g_t[:, DG, :] = 1.0
    p_arr = np.arange(64)
    tA = 8 * (p_arr // 4) + (p_arr % 4)
    g_t2 = np.concatenate([g_t[tA], g_t[tA + 4]], axis=2)  # [64, 9, 2*keyc]
    out["g_t2"] = np.ascontiguousarray(g_t2.astype(BF16))
    # Upack + residual/mask smalls
    upack = np.zeros((128, 128 * NPAIR), np.float64)
    small = np.zeros((128, 8), np.float32)
    cfq = cf[:, qs]                                      # [B,QL,S,C]
    for t in range(T):
        b, r = divmod(t, QL * S)
        ql, sq = divmod(r, S)
        P, u = divmod(t, 32)
        cg, s = u % 4, u // 4
        for c in range(C):
            fq = cfq[b, ql, sq, c]
            u_vec = G[b, c] @ _lag(np.array([fq]), nodes[b, c], cw)[0]
            row = 32 * cg + 4 * s + c
            upack[32 * c:32 * c + 32, 128 * P + row] = u_vec
            small[row, P] = fq
            small[row, 4 + P] = mask[b, core * QL + ql, sq]
    out["upack"] = upack.astype(BF16)
    out["small128"] = small
    return out


def _build_program(keyc):
    KEY = keyc   # key-compacted free dimension (shadows the module constant)
    KEYF = 512   # PSUM half-stride: matmul outputs must not cross a bank
    from contextlib import ExitStack
    import concourse.bass as bass
    import concourse.tile as tile
    import concourse.mybir as mybir
    from concourse import bacc
    import bass_rust

    f32 = mybir.dt.float32
    bf16 = mybir.dt.bfloat16
    AF = mybir.ActivationFunctionType
    ALU = mybir.AluOpType

    nc = bacc.Bacc("TRN2", target_bir_lowering=False, debug=False,
                   enable_asserts=False, num_devices=NCORE)

    din = {}
    for name, shape, dt in (
        ("g_t2", [64, DG + 1, 2 * KEY], bf16),
        ("w1g", [128, 128], bf16),
        ("w2full", [128, 128], bf16),
        ("w3g", [128, 256], bf16),
        ("bias128", [128, 8], f32),
        ("upack", [128, 128 * NPAIR], bf16),
        ("psin2", [128, B * KEY], bf16),
        ("psid2", [128, B * KEY], bf16),
        ("small128", [128, 8], f32),
    ):
        din[name] = nc.dram_tensor(name, shape, dt, kind="ExternalInput").ap()
    dout = nc.dram_tensor("out128", [128, NPAIR], f32, kind="ExternalOutput").ap()

    with tile.TileContext(nc) as tc, ExitStack() as ctx:
        const = ctx.enter_context(tc.tile_pool(name="const", bufs=1))
        work = ctx.enter_context(tc.tile_pool(name="work", bufs=2))
        ps = ctx.enter_context(tc.tile_pool(name="ps", bufs=1, space="PSUM"))
        ep = ctx.enter_context(tc.tile_pool(name="ep", bufs=2))

        # --- constants to SBUF (w1g + first g tiles first: shortest path
        # to the first L1 matmul; the warm-up burst uses memset zeros) ---
        w1g_s = const.tile([128, 128], bf16, name="w1g_s")
        nc.sync.dma_start(w1g_s[:], din["w1g"][:])
        upack_s = const.tile([128, 128 * NPAIR], bf16, name="upack_s")
        w2full_s = const.tile([128, 128], bf16, name="w2full_s")
        bias128_s = const.tile([128, 8], f32, name="bias128_s")
        nc.sync.dma_start(bias128_s[:], din["bias128"][:])
        small128_s = const.tile([128, 8], f32, name="small128_s")
        nc.sync.dma_start(small128_s[:], din["small128"][:])
        # big consts DMA'd from inside the loop (after the first g tiles)
        w3g_s = const.tile([128, 256], bf16, name="w3g_s")
        psin2_s = const.tile([128, B * KEY], bf16, name="psin2_s")
        psid2_s = const.tile([128, B * KEY], bf16, name="psid2_s")
        logits_all = const.tile([128, NPAIR * KEY], f32, name="logits_all")
        pfac_s = const.tile([128, NPAIR * 2 * KEY], f32, name="pfac_s")
        out_s = const.tile([128, NPAIR], f32, name="out_s")

        # --- HAM warm-up: 15 dense FULL-ARRAY (K=128, M=128) matmuls.  The
        # clock gate only *latches* 8/8 under high array occupancy, but any
        # activity then *keeps* it warm — so one full burst up front makes
        # the whole tile-packed pipeline run at 2.4 GHz. ---
        gtb = [const.tile([128, 2 * KEY], bf16, name=f"gtb{i}") for i in range(3)]
        dummy_src = const.tile([128, KEY], bf16, name="dummy_src")
        nc.vector.memset(dummy_src[:], 0.0)
        for i in range(3):
            nc.vector.memset(gtb[i][:], 0.0)

        scratch = ps.tile([128, KEY], f32, tag="warm", bufs=1, name="scratch")
        for _ in range(2):
            nc.tensor.matmul(scratch[:], dummy_src[:, 0:128], dummy_src[:],
                             start=True, stop=True)

        def warm_fill(n):
            # Dependency-free full-array (K=128) short matmuls: the PE runs
            # them whenever it would otherwise idle, so the HAM activity
            # monitor never sees an idle window and 2.4 GHz persists.
            for _ in range(n):
                nc.tensor.matmul(scratch[:], dummy_src[:, 0:128],
                                 dummy_src[:], start=True, stop=True)

        # --- E_y factor maps: 8 matmuls + DVE copies to SBUF, emitted one
        # per early pipeline step (own 1-bank psum tag; fills PE gaps). ---
        def fac_stage(k):
            P, half = divmod(k, 2)
            b = P // (NPAIR // B)
            fsrc = psin2_s if half == 0 else psid2_s
            nc.tensor.matmul(scratch[:], upack_s[:, 128 * P:128 * (P + 1)],
                             fsrc[:, b * KEY:(b + 1) * KEY],
                             start=True, stop=True, tile_position=(0, 0))
            nc.vector.tensor_copy(
                pfac_s[:, (2 * P + half) * KEY:(2 * P + half + 1) * KEY],
                scratch[:])

        gts = {}
        ps1s = {}
        h1s = {}
        ps2s = {}
        h2s = {}
        ps3s = {}
        state = {"last": None}

        def dma_stage(p):
            gt = gtb[p % 3]
            nc.sync.dma_start(gt[0:DG + 1, :], din["g_t2"][p])
            gts[p] = gt

        def l1_stage(p):
            gt = gts.pop(p)
            pA = ps.tile([128, 2 * KEYF], f32, tag="psL1", bufs=1, name="pA")
            nc.tensor.matmul(pA[:, 0:KEY], w1g_s[:], gt[:, 0:KEY],
                             start=True, stop=True)
            nc.tensor.matmul(pA[:, KEYF:KEYF + KEY], w1g_s[:],
                             gt[:, KEY:2 * KEY], start=True, stop=True)
            ps1s[p] = pA

        def s1_stage(p):
            pA = ps1s.pop(p)
            h1 = work.tile([128, 2 * KEY], bf16, tag="h1", bufs=2, name="h1")
            nc.scalar.activation(
                h1[:].rearrange("p (h k) -> p h k", h=2),
                pA[:].rearrange("p (h k) -> p h k", h=2)[:, :, 0:KEY],
                AF.Silu, bias=0.0)
            h1s[p] = h1

        def l2_stage(p):
            h1 = h1s.pop(p)
            pB = ps.tile([128, 2 * KEYF], f32, tag="psH2", bufs=1, name="pB")
            for half in range(2):
                nc.tensor.matmul(
                    pB[:, half * KEYF:half * KEYF + KEY],
                    w2full_s[:],
                    h1[:, half * KEY:(half + 1) * KEY],
                    start=True, stop=True)
            ps2s[p] = pB

        def s2_stage(p):
            pB = ps2s.pop(p)
            h2 = work.tile([128, 2 * KEY], bf16, tag="h2", bufs=2, name="h2")
            nc.scalar.activation(
                h2[:].rearrange("p (h k) -> p h k", h=2),
                pB[:].rearrange("p (h k) -> p h k", h=2)[:, :, 0:KEY],
                AF.Silu, bias=bias128_s[:, 0:1])
            h2s[p] = h2

        def l3_stage(p):
            rho = p % 4
            chunk = p // 4
            P = chunk // 4
            s0 = 2 * (chunk % 4)
            if p % 16 == 0:
                ps3s[P] = ps.tile([128, KEY], f32, tag="psL3", bufs=2,
                                  name="ps3")
            ps3 = ps3s[P]
            h2 = h2s.pop(p)
            for half in range(2):
                s = s0 + half
                nc.tensor.matmul(
                    ps3[32 * rho:32 * rho + 32, :],
                    w3g_s[:, 32 * s:32 * s + 32],
                    h2[:, half * KEY:(half + 1) * KEY],
                    start=(s == 0), stop=(s == 7), tile_position=(0, 32 * rho))
            if p % 16 == 15:
                ps3s.pop(P)
                h = nc.scalar.activation(
                    logits_all[:, P * KEY:(P + 1) * KEY], ps3[:], AF.Silu,
                    bias=bias128_s[:, 4:5])
                state["last"] = h.ins

        # ============ phase 1: 5-deep software pipeline over 64 pairs ======
        # silu1 leads silu2 by one pair so the L2 matmuls of pair p hide
        # under silu1(p+1) instead of stalling the ACT engine.
        dma_stage(0)
        dma_stage(1)
        nc.sync.dma_start(w2full_s[:], din["w2full"][:])
        nc.sync.dma_start(upack_s[:], din["upack"][:])
        for step in range(64 + 4):
            if 2 <= step < 64:
                dma_stage(step)
            if 3 <= step <= 66:
                l2_stage(step - 3)
            if 1 <= step <= 64:
                l1_stage(step - 1)
            if 2 <= step <= 65:
                s1_stage(step - 2)
            if 3 <= step <= 66:
                s2_stage(step - 3)
            if step >= 4:
                l3_stage(step - 4)
            if step == 1:
                nc.sync.dma_start(w3g_s[:], din["w3g"][:])
            if step == 2:
                nc.sync.dma_start(psin2_s[:], din["psin2"][:])
            if step == 3:
                nc.sync.dma_start(psid2_s[:], din["psid2"][:])
            if 10 <= step <= 17:
                fac_stage(step - 10)
            warm_fill(1)
        last_silu = state["last"]

        # ============ phase 2: exp + aggregate against factor maps =========
        numden = const.tile([128, 8], f32, name="numden")
        for P in range(NPAIR):
            e = ep.tile([128, KEY], f32, tag="e", bufs=3, name="e")
            h = nc.scalar.activation(e[:], logits_all[:, P * KEY:(P + 1) * KEY],
                                     AF.Exp)
            bass_rust.add_dep_helper(h.ins, last_silu,
                                     reason="act-table phase barrier")
            scrN = ep.tile([128, KEY], f32, tag="scrN", name="scrN")
            scrD = ep.tile([128, KEY], f32, tag="scrD", name="scrD")
            pn = pfac_s[:, P * 2 * KEY:P * 2 * KEY + KEY]
            pd = pfac_s[:, P * 2 * KEY + KEY:(P + 1) * 2 * KEY]
            nc.vector.affine_mul_reduce(scrN[:], numden[:, P:P + 1], e[:], pn,
                                        1.0, 0.0)
            nc.vector.affine_mul_reduce(scrD[:], numden[:, 4 + P:5 + P], e[:],
                                        pd, 1.0, 0.0)
        rden = ep.tile([128, 4], f32, tag="rden", name="rden")
        nc.vector.reciprocal(rden[:], numden[:, 4:8])
        agg = ep.tile([128, 4], f32, tag="agg", name="agg")
        nc.vector.tensor_mul(agg[:], numden[:, 0:4], rden[:])
        res = ep.tile([128, 4], f32, tag="res", name="res")
        nc.vector.tensor_add(res[:], agg[:], small128_s[:, 0:4])
        nc.vector.tensor_mul(out_s[:], res[:], small128_s[:, 4:8])
        nc.sync.dma_start(dout[:], out_s[:])

    nc.compile()
    return nc


def _get_program(keyc):
    global _PROG
    if _PROG is None or _PROG[0] != keyc:
        _PROG = (keyc, _build_program(keyc))
    return _PROG[1]


def _build_inmaps(inp):
    mask = np.asarray(inp["mask"])
    kept, valid, keyc = _kept_indices(mask)
    gl, aux = _pack_globals(inp, kept, valid, keyc)
    in_maps = []
    for core in range(NCORE):
        m = dict(gl)
        m.update(_pack_core(core, inp, aux, kept, keyc))
        in_maps.append({k: np.ascontiguousarray(v) for k, v in m.items()})
    return in_maps, keyc


def kernel(**inputs) -> np.ndarray:
    from concourse.bass_utils import run_bass_kernel_spmd

    inp = {k: np.asarray(v) for k, v in inputs.items()}
    w_out = np.asarray(inp["w_out"], np.float32)
    in_maps, keyc = _build_inmaps(inp)
    nc = _get_program(keyc)
    res = run_bass_kernel_spmd(nc, in_maps, core_ids=list(range(NCORE)))

    cf_out = np.zeros((B, N, S, C), np.float32)
    for core in range(NCORE):
        OUT = res.results[core]["out128"]                  # [128, NPAIR]
        # row = 32*cg + 4*s + c, col = P;  t = 32*P + 4*s + cg
        arr = OUT.reshape(4, 8, C, NPAIR).transpose(3, 1, 0, 2).reshape(T, C)
        arr = arr.reshape(B, QL, S, C)
        cf_out[:, core * QL:(core + 1) * QL] = arr
    return (cf_out @ w_out.T).astype(np.float32)
